# revision 1
# baseline (speedup 1.0000x reference)
import sys
if "/opt/trn_rl_repo" not in sys.path:
    sys.path.insert(0, "/opt/trn_rl_repo")
"""Builder for the MoE Bass/Tile kernel (shared by kernel.py and test scripts).

Per-core program: x shard [NTOK, H] -> 2 MoE layers -> y [NTOK, H].
Data-parallel over tokens across 8 cores; all weights replicated.

Layer dataflow:
  Phase A (per 128-token tile):
    - DMA x tile [128, H]
    - LN stats on DVE/ACT (mean via reduce, var via ACT Square+accum)
    - z = (x-mu)*rsig in one DVE tensor_scalar
    - PE-transpose z into resident zT tiles [128(H-chunk), NTOK], applying
      ln_g/ln_b per H-element (per-partition scale/bias in zT layout) on ACT
    - router logits via full-fp32 PE matmuls (exact; top-2 selection is
      discontinuous so router must match the fp32 reference closely)
    - top-2 renormalized softmax weights w [128, E] via DVE/ACT chain
  Phase B (per 512-wide output tile, per expert):
    - stream We chunks, accumulate z @ We over K into per-token-tile PSUM banks
    - drain: ACT scales by w[:, e] (per-partition scalar), DVE accumulates
    - acc initialized by DMA of x slice (residual) + be mix via tiny K=E matmul
"""

import numpy as np

import concourse.bass as bass
import concourse.bacc as bacc
import concourse.mybir as mybir
import concourse.tile as tile
from concourse import masks

F32 = mybir.dt.float32
F32R = mybir.dt.float32r
AF = mybir.ActivationFunctionType
ALU = mybir.AluOpType
AX = mybir.AxisListType

LN_EPS = 1e-5


def build_moe_kernel(NTOK, H, E, L, HO=512, expert_dtype="f32r", surrogate=True):
    """Returns compiled nc. Inputs: x [NTOK,H], ln_g/ln_b [L,H], Wr [L,H,E],
    br [L,E], We [L,E,H,H], be [L,E,H]. Output: y [NTOK,H]."""
    assert NTOK % 128 == 0 and H % 128 == 0 and H % HO == 0
    KT = H // 128          # number of 128-row contraction chunks
    NI = NTOK // 128       # number of 128-token tiles
    NHO = H // HO          # number of output column tiles
    KB = 4 if KT % 4 == 0 else 1   # K-chunks batched per weight DMA

    use_f32r = expert_dtype == "f32r"
    WDT = F32R if use_f32r else F32
    nc = bacc.Bacc("TRN2", target_bir_lowering=False, debug=False)
    x_d = nc.declare_dram_parameter("x", [NTOK, H], F32, False)
    lng_d = nc.declare_dram_parameter("ln_g", [L, H], F32, False)
    lnb_d = nc.declare_dram_parameter("ln_b", [L, H], F32, False)
    wr_d = nc.declare_dram_parameter("Wr", [L, H, E], F32, False)
    br_d = nc.declare_dram_parameter("br", [L, E], F32, False)
    we_d = nc.declare_dram_parameter("We", [L, E, H, H], WDT, False)
    be_d = nc.declare_dram_parameter("be", [L, E, H], F32, False)
    NU = 4 + 4 * E + E    # surrogate projection columns: A/g1 | We@A per e | rowmean We per e
    if surrogate:
        assert L == 2
        uc_d = nc.declare_dram_parameter("Ucomb", [H, NU], F32, False)
        rc_d = nc.declare_dram_parameter("rconst", [8, E], F32, False)
    y_d = nc.declare_dram_parameter("y", [NTOK, H], F32, True)
    x1_d = nc.dram_tensor("x1_scratch", [NTOK, H], F32)

    with tile.TileContext(nc) as tc:
        with (
            tc.tile_pool(name="const", bufs=1) as constp,
            tc.tile_pool(name="lcon", bufs=2) as lconp,      # per-layer consts
            tc.tile_pool(name="xin", bufs=2) as xp,
            tc.tile_pool(name="zT", bufs=1) as ztp,
            tc.tile_pool(name="zf", bufs=1) as zfp,          # transient f32 zT chunks (router)
            tc.tile_pool(name="small", bufs=4 * NI) as smp,
            tc.tile_pool(name="wrout", bufs=3 * NI) as wp,   # router weights w
            tc.tile_pool(name="wch", bufs=3) as wchp,        # streamed We chunks
            tc.tile_pool(name="acc", bufs=NI) as accp,
            tc.tile_pool(name="tmp", bufs=2) as tmpp,
            tc.tile_pool(name="ps", bufs=8, space="PSUM") as psp,
        ):
            ident = constp.tile([128, 128], F32)
            masks.make_identity(nc, ident[:])
            eps_t = constp.tile([128, 1], F32)
            nc.gpsimd.memset(eps_t[:], LN_EPS)

            for l in range(L):
                x_src = x_d.ap() if l == 0 else x1_d.ap()
                dst = y_d.ap() if l == L - 1 else x1_d.ap()

                # ---- per-layer constants ----
                g_sb = lconp.tile([128, KT], F32, tag="g")
                nc.sync.dma_start(g_sb[:], lng_d.ap()[l].rearrange("(k p) -> p k", p=128))
                b_sb = lconp.tile([128, KT], F32, tag="b")
                nc.sync.dma_start(b_sb[:], lnb_d.ap()[l].rearrange("(k p) -> p k", p=128))
                wr_sb = lconp.tile([128, KT, E], F32, tag="wr")
                nc.sync.dma_start(wr_sb[:], wr_d.ap()[l].rearrange("(k p) e -> p k e", p=128))
                # br broadcast to all partitions (DRAM-side partition step 0)
                br_bc = lconp.tile([128, E], F32, tag="br")
                nc.sync.dma_start(br_bc[:], br_d.ap()[l].unsqueeze(0).broadcast_to((128, E)))
                if surrogate and l == 0:
                    u_sb = lconp.tile([128, KT, NU], F32, tag="uc", bufs=1)
                    nc.sync.dma_start(u_sb[:], uc_d.ap().rearrange("(k p) u -> p k u", p=128))
                if surrogate and l == 1:
                    rc_bc = lconp.tile([128, 8 * E], F32, tag="rc", bufs=1)
                    nc.sync.dma_start(rc_bc[:], rc_d.ap().rearrange("a b -> (a b)").unsqueeze(0).broadcast_to((128, 8 * E)))

                # ---- Phase A (software-pipelined: LN of tile i overlaps
                # transposes/router of tile i-1) ----
                zT = ztp.tile([128, KT, NTOK], WDT, tag="zT", name="zT")
                w_tiles = []
                wT_tiles = []
                xts = [None] * NI
                rsigs = [None] * NI
                if l == 0:
                    zu_tiles, mu_c, sd_c, w0_tiles = [], [], [], []
                for ii in range(NI + 1):
                    if ii < NI:
                        i = ii
                        tsl = slice(i * 128, (i + 1) * 128)
                        xt = xp.tile([128, H], F32, tag="x")
                        nc.sync.dma_start(xt[:], x_src[tsl, :])
                        xts[i] = xt

                        s1 = smp.tile([128, 1], F32, tag="s")
                        nc.vector.tensor_reduce(s1[:], xt[:], AX.X, ALU.add)
                        mu = smp.tile([128, 1], F32, tag="muc", bufs=2 * NI)
                        nc.vector.tensor_scalar_mul(mu[:], s1[:], 1.0 / H)

                        SQC = min(HO, H)
                        nsq = H // SQC
                        s2p = smp.tile([128, max(nsq, 2)], F32, tag="sp")
                        for c in range(nsq):
                            sqps = psp.tile([128, SQC], F32, tag="ps", name="sqps")
                            nc.scalar.activation(sqps[:], xt[:, c * SQC:(c + 1) * SQC],
                                                 AF.Square, accum_out=s2p[:, c:c + 1])
                        s2 = smp.tile([128, 1], F32, tag="s")
                        nc.vector.tensor_reduce(s2[:], s2p[:, :nsq], AX.X, ALU.add)

                        ex2 = smp.tile([128, 1], F32, tag="s")
                        nc.vector.tensor_scalar_mul(ex2[:], s2[:], 1.0 / H)
                        musq = smp.tile([128, 1], F32, tag="s")
                        nc.vector.tensor_mul(musq[:], mu[:], mu[:])
                        var = smp.tile([128, 1], F32, tag="s")
                        nc.vector.tensor_sub(var[:], ex2[:], musq[:])
                        sd = smp.tile([128, 1], F32, tag="sdc", bufs=2 * NI)
                        nc.scalar.activation(sd[:], var[:], AF.Sqrt, bias=eps_t[:])
                        rsig = smp.tile([128, 1], F32, tag="s")
                        nc.vector.reciprocal(rsig[:], sd[:])
                        rsigs[i] = rsig
                        if surrogate and l == 0:
                            mu_c.append(mu)
                            sd_c.append(sd)

                        # z = (x - mu) * rsig, in place over the x tile
                        nc.vector.tensor_scalar(xt[:], xt[:], mu[:], rsig[:],
                                                ALU.subtract, ALU.mult)

                    if ii > 0:
                        i = ii - 1
                        tsl = slice(i * 128, (i + 1) * 128)
                        xt = xts[i]
                        # transpose into zT (f32r, experts) and, when the plain
                        # router runs, zf (exact f32 copy for it)
                        plain_router = not (surrogate and l == 1)
                        TB = 4 if KT % 4 == 0 else 1
                        if plain_router:
                            zf = zfp.tile([128, KT, 128], F32, tag="zf", name="zf")
                        for kb in range(KT // TB):
                            pt = psp.tile([128, TB, 128], F32, tag="ps", name="pt")
                            for j in range(TB):
                                k = kb * TB + j
                                nc.tensor.transpose(pt[:, j, :],
                                                    xt[:, k * 128:(k + 1) * 128], ident[:])
                            for j in range(TB):
                                k = kb * TB + j
                                nc.scalar.activation(zT[:, k, tsl], pt[:, j, :], AF.Identity,
                                                     scale=g_sb[:, k:k + 1],
                                                     bias=b_sb[:, k:k + 1])
                                if plain_router:
                                    nc.vector.tensor_scalar(zf[:, k, :], pt[:, j, :],
                                                            g_sb[:, k:k + 1], b_sb[:, k:k + 1],
                                                            ALU.mult, ALU.add)

                        if plain_router:
                            # router logits: full fp32 matmul (exact)
                            lp = psp.tile([128, E], F32, tag="ps")
                            for k in range(KT):
                                nc.tensor.matmul(lp[:], zf[:, k, :], wr_sb[:, k, :],
                                                 start=(k == 0), stop=(k == KT - 1))
                            ls = wp.tile([128, E], F32, tag="w")
                            nc.vector.tensor_add(ls[:], lp[:], br_bc[:])
                        if surrogate and l == 0:
                            # layer-2 router projections: zU = z @ [A/g1 | We@A | rowmean-We]
                            pu = psp.tile([128, NU], F32, tag="ps")
                            for k in range(KT):
                                nc.tensor.matmul(pu[:], zf[:, k, :], u_sb[:, k, :],
                                                 start=(k == 0), stop=(k == KT - 1))
                            zu = wp.tile([128, NU], F32, tag="zu", bufs=2 * NI)
                            nc.vector.tensor_copy(zu[:], pu[:])
                            zu_tiles.append(zu)
                        if surrogate and l == 1:
                            # exact-reference layer-2 logits, bypassing the lossy
                            # z@We path:  logits = rsig2*(x1@A - mu(x1)*sumA) + bW
                            zu = zu_tiles[i]
                            w0 = w0_tiles[i]
                            mu0 = mu_c[i]
                            sd0 = sd_c[i]
                            t1 = wp.tile([128, E], F32, tag="w")
                            nc.vector.tensor_sub(t1[:], zu[:, 0:4], rc_bc[:, 0:4])
                            t2 = wp.tile([128, E], F32, tag="w")
                            nc.vector.tensor_scalar_mul(t2[:], t1[:], sd0[:])
                            t3 = wp.tile([128, E], F32, tag="w")
                            nc.vector.tensor_scalar_mul(t3[:], rc_bc[:, 4:8], mu0[:])
                            xA = wp.tile([128, E], F32, tag="w")
                            nc.vector.tensor_add(xA[:], t2[:], t3[:])
                            u16 = wp.tile([128, 4 * E], F32, tag="w16", bufs=4)
                            nc.vector.tensor_add(u16[:], zu[:, 4:4 + 4 * E],
                                                 rc_bc[:, 8:8 + 4 * E])
                            macc = None
                            for e in range(E):
                                te = wp.tile([128, E], F32, tag="w", name="te")
                                nc.vector.tensor_scalar_mul(te[:], u16[:, 4 * e:4 * e + 4],
                                                            w0[:, e:e + 1])
                                if macc is None:
                                    macc = te
                                else:
                                    macc2 = wp.tile([128, E], F32, tag="w", name="macc2")
                                    nc.vector.tensor_add(macc2[:], macc[:], te[:])
                                    macc = macc2
                            x1A = wp.tile([128, E], F32, tag="w")
                            nc.vector.tensor_add(x1A[:], xA[:], macc[:])
                            # mu(x1) = mu0 + sum_e w0_e*(zMe + mean_be)
                            m4 = wp.tile([128, E], F32, tag="w")
                            nc.vector.tensor_add(m4[:], zu[:, 4 + 4 * E:4 + 5 * E],
                                                 rc_bc[:, 24:28])
                            m4w = wp.tile([128, E], F32, tag="w")
                            nc.vector.tensor_mul(m4w[:], m4[:], w0[:])
                            ms = smp.tile([128, 1], F32, tag="s")
                            nc.vector.tensor_reduce(ms[:], m4w[:], AX.X, ALU.add)
                            mux1 = smp.tile([128, 1], F32, tag="s")
                            nc.vector.tensor_add(mux1[:], mu0[:], ms[:])
                            s4 = wp.tile([128, E], F32, tag="w")
                            nc.vector.tensor_scalar_mul(s4[:], rc_bc[:, 4:8], mux1[:])
                            l0 = wp.tile([128, E], F32, tag="w")
                            nc.vector.tensor_sub(l0[:], x1A[:], s4[:])
                            l1 = wp.tile([128, E], F32, tag="w")
                            nc.vector.tensor_scalar_mul(l1[:], l0[:], rsigs[i][:])
                            ls = wp.tile([128, E], F32, tag="w")
                            nc.vector.tensor_add(ls[:], l1[:], rc_bc[:, 28:32])

                        # top-2 renormalized softmax
                        m1 = smp.tile([128, 1], F32, tag="s")
                        nc.vector.tensor_reduce(m1[:], ls[:], AX.X, ALU.max)
                        nm1 = smp.tile([128, 1], F32, tag="s")
                        nc.vector.tensor_scalar_mul(nm1[:], m1[:], -1.0)
                        selmax = wp.tile([128, E], F32, tag="w")
                        nc.vector.tensor_scalar(selmax[:], ls[:], m1[:], 1e30,
                                                ALU.is_ge, ALU.mult)
                        lmsk = wp.tile([128, E], F32, tag="w")
                        nc.vector.tensor_sub(lmsk[:], ls[:], selmax[:])
                        m2 = smp.tile([128, 1], F32, tag="s")
                        nc.vector.tensor_reduce(m2[:], lmsk[:], AX.X, ALU.max)
                        sel2 = wp.tile([128, E], F32, tag="w")
                        nc.vector.tensor_scalar(sel2[:], ls[:], m2[:], None, ALU.is_ge)
                        et = wp.tile([128, E], F32, tag="w")
                        nc.scalar.activation(et[:], ls[:], AF.Exp, bias=nm1[:])
                        ew = wp.tile([128, E], F32, tag="w")
                        nc.vector.tensor_mul(ew[:], et[:], sel2[:])
                        ssum = smp.tile([128, 1], F32, tag="s")
                        nc.vector.tensor_reduce(ssum[:], ew[:], AX.X, ALU.add)
                        rs = smp.tile([128, 1], F32, tag="s")
                        nc.vector.reciprocal(rs[:], ssum[:])
                        w_t = wp.tile([128, E], F32, tag="w")
                        nc.vector.tensor_scalar_mul(w_t[:], ew[:], rs[:])
                        w_tiles.append(w_t)
                        if surrogate and l == 0:
                            w0_tiles.append(w_t)

                        # wT for the be-mix matmul: [E, 128]
                        pw = psp.tile([E, 128], F32, tag="ps")
                        nc.tensor.transpose(pw[:], w_t[:], ident[:])
                        wT = wp.tile([E, 128], F32, tag="wT")
                        nc.vector.tensor_copy(wT[:], pw[:])
                        wT_tiles.append(wT)

                # ---- Phase B ----
                for ho in range(NHO):
                    osl = slice(ho * HO, (ho + 1) * HO)
                    be_sb = lconp.tile([E, HO], F32, tag="be")
                    nc.sync.dma_start(be_sb[:], be_d.ap()[l][:, osl])
                    accs = []
                    for i in range(NI):
                        tsl = slice(i * 128, (i + 1) * 128)
                        acc = accp.tile([128, HO], F32, tag="acc")
                        nc.sync.dma_start(acc[:], x_src[tsl, osl])
                        # be mix: acc += w_i @ be[l][:, osl]
                        pbe = psp.tile([128, HO], F32, tag="ps")
                        nc.tensor.matmul(pbe[:], wT_tiles[i][:], be_sb[:],
                                         start=True, stop=True)
                        nc.vector.tensor_add(acc[:], acc[:], pbe[:])
                        accs.append(acc)

                    for e in range(E):
                        wmat = we_d.ap()[l, e].rearrange(
                            "(kb j p) n -> p kb j n", p=128, j=KB)
                        wcs = []
                        for kb in range(KT // KB):
                            wc = wchp.tile([128, KB, HO], WDT, tag="wch")
                            nc.sync.dma_start(wc[:], wmat[:, kb, :, osl])
                            wcs.append(wc)
                        pbs = [psp.tile([128, HO], F32, tag="ps", name="pbs") for _ in range(NI)]
                        for k in range(KT):
                            kb, j = divmod(k, KB)
                            rhs = wcs[kb][:, j, :]
                            for i in range(NI):
                                tsl = slice(i * 128, (i + 1) * 128)
                                nc.tensor.matmul(pbs[i][:], zT[:, k, tsl], rhs,
                                                 start=(k == 0), stop=(k == KT - 1))
                        for i in range(NI):
                            tm = tmpp.tile([128, HO], F32, tag="tmp")
                            nc.scalar.activation(tm[:], pbs[i][:], AF.Copy,
                                                 scale=w_tiles[i][:, e:e + 1])
                            nc.vector.tensor_add(accs[i][:], accs[i][:], tm[:])

                    for i in range(NI):
                        tsl = slice(i * 128, (i + 1) * 128)
                        nc.sync.dma_start(dst[tsl, osl], accs[i][:])

    nc.compile()
    return nc


def moe_reference_np(x, ln_g, ln_b, Wr, br, We, be, dtype=np.float32):
    """Numpy mirror of reference.py (for small-size validation)."""
    x = x.astype(dtype)
    L = ln_g.shape[0]
    N, H = x.shape
    for l in range(L):
        mu = x.mean(-1, keepdims=True, dtype=dtype)
        var = x.var(-1, keepdims=True, dtype=dtype)
        z = (x - mu) / np.sqrt(var + LN_EPS) * ln_g[l] + ln_b[l]
        logits = z @ Wr[l] + br[l]
        probs = np.exp(logits - logits.max(-1, keepdims=True))
        probs /= probs.sum(-1, keepdims=True)
        top2 = np.argsort(-logits, -1, kind="stable")[:, :2]
        mask = np.zeros_like(probs)
        np.put_along_axis(mask, top2, np.take_along_axis(probs, top2, -1), -1)
        w = mask / np.clip(mask.sum(-1, keepdims=True), 1e-8, None)
        outs = np.einsum("th,ehd->ted", z, We[l]) + be[l]
        x = x + np.einsum("te,ted->td", w, outs)
    return x


# ======== kernel entry points ========

N_CORES = 8
B, T, H, E, L = 4, 2048, 3072, 4, 2
NTOK_TOTAL = B * T
NTOK = NTOK_TOTAL // N_CORES

_nc_cache = {}


def _get_nc():
    if "nc" not in _nc_cache:
        _nc_cache["nc"] = build_moe_kernel(NTOK, H, E, L, 512)
    return _nc_cache["nc"]


def _round_fp22(a):
    """Round f32 to fp22 (13 explicit mantissa bits, RNE-ish) so the on-chip
    f32r conversion of We is an exact identity regardless of HW rounding mode."""
    u = np.ascontiguousarray(a, np.float32).view(np.uint32)
    return ((u + np.uint32(0x200)) & np.uint32(0xFFFFFC00)).view(np.float32)


def _surrogate_consts(ln_g, ln_b, Wr, br, We, be):
    """Host fp64 precompute for the exact layer-2 router surrogate:
    logits2 = rsig2*(x1@A - mu(x1)*sumA) + b2@Wr2 + br2 with
    x1@A = x@A + sum_e w_e (z@(We@A) + be@A)."""
    g1 = ln_g[0].astype(np.float64); b1 = ln_b[0].astype(np.float64)
    g2 = ln_g[1].astype(np.float64); b2 = ln_b[1].astype(np.float64)
    A = g2[:, None] * Wr[1].astype(np.float64)          # [H, E]
    A1 = A / g1[:, None]
    cols = [A1]
    for e in range(E):
        cols.append(We[0, e].astype(np.float64) @ A)    # [H, E]
    for e in range(E):
        cols.append(We[0, e].astype(np.float64).mean(axis=1)[:, None])
    Ucomb = np.concatenate(cols, axis=1).astype(np.float32)  # [H, 4+4E+E]
    rconst = np.zeros((8, E), np.float64)
    rconst[0] = b1 @ A1
    rconst[1] = A.sum(0)
    for e in range(E):
        rconst[2 + e] = be[0, e].astype(np.float64) @ A
    rconst[6] = [be[0, e].mean(dtype=np.float64) for e in range(E)]
    rconst[7] = b2 @ Wr[1].astype(np.float64) + br[1]
    return Ucomb, rconst.astype(np.float32)


def _make_in_maps(x, ln_g, ln_b, Wr, br, We, be):
    xf = np.ascontiguousarray(x.reshape(NTOK_TOTAL, H), dtype=np.float32)
    Ucomb, rconst = _surrogate_consts(ln_g, ln_b, Wr, br, We, be)
    shared = {
        "ln_g": np.ascontiguousarray(ln_g, np.float32),
        "ln_b": np.ascontiguousarray(ln_b, np.float32),
        "Wr": np.ascontiguousarray(Wr, np.float32),
        "br": np.ascontiguousarray(br, np.float32),
        "We": _round_fp22(We),
        "be": np.ascontiguousarray(be, np.float32),
        "Ucomb": Ucomb,
        "rconst": rconst,
    }
    return [
        {"x": xf[c * NTOK:(c + 1) * NTOK], **shared}
        for c in range(N_CORES)
    ]


def kernel(x, ln_g, ln_b, Wr, br, We, be):
    from concourse.bass_utils import run_bass_kernel_spmd
    nc = _get_nc()
    in_maps = _make_in_maps(x, ln_g, ln_b, Wr, br, We, be)
    res = run_bass_kernel_spmd(nc, in_maps, core_ids=list(range(N_CORES)))
    y = np.concatenate([res.results[c]["y"] for c in range(N_CORES)], axis=0)
    return y.reshape(B, T, H).astype(np.float32)


def run_profiled(inputs):
    from concourse.bass_utils import run_bass_kernel_spmd
    nc = _get_nc()
    in_maps = _make_in_maps(**inputs)
    return run_bass_kernel_spmd(nc, in_maps, core_ids=list(range(N_CORES)),
                                trace=True)

